# revision 4
# baseline (speedup 1.0000x reference)
"""Causal self-attention TRN2 kernel.

Problem: B=4, T=2048, C=1024, H=16 heads, Dh=64, fp32 I/O.

Sharding: 8 cores = 4 batches x 2 head-groups (8 heads each). Each core
computes QKV projection for its head-group, causal attention, and a partial
output projection; the host sums the two partials per batch and adds b_out.

Per-core layout (all matmul operands fp16; PSUM accumulation fp32):
  - xT [C, T] fp16 (host-transposed), Wq/Wk/Wv [C, 512] fp16, Wo [512, C] fp16
  - QT/KT [512, T] fp16 stored as 4 head-pair tiles [128, T] (partition = 2x64 dh)
  - V [T, 520] fp16 stored as 16 t-block tiles [128, 8, 65]: per head 64 V cols
    + a ones column (the AV matmul's ones column accumulates softmax row-sums)
  - scores computed transposed: ST [keys 128, queries 512] = KTh_blk.T @ QTh_chunk
    so exp(ST) feeds the AV matmul directly (no P transpose needed)
  - AV: OT [65, 512] += Vaug[128, 65].T @ P[128, 512]; row 64 = softmax sums
  - normalize: recip of sum row, partition-broadcast via K=1 fp32 matmul
    (ones[1,64].T @ recip[1,512]), fused into the PSUM->SBUF eviction multiply
  - out proj: yT [C, T] fp32 partial = Wo_hp.T @ OT_hp accumulated over head pairs

loop_iters>1 wraps the whole body in a tc.For_i hardware loop (timing only).
"""

import numpy as np

import concourse.bacc as bacc
import concourse.mybir as mybir
import concourse.tile as tile
from concourse import bass_utils

F32 = mybir.dt.float32
F16 = mybir.dt.float16
AF = mybir.ActivationFunctionType

B, T, C = 4, 2048, 1024
H, DH = 16, 64
HPC = 512          # head dims per core (8 heads x 64)
NHP = 4            # head pairs per core
NC_CHUNKS = C // 128   # 8 contraction chunks
NTS = T // 512     # 4 t-chunks of 512
NTB = T // 128     # 16 t-blocks of 128
SCALE = 1.0 / np.sqrt(DH)

_cache = {}


def _build(loop_iters=1):
    key = ("nc", loop_iters)
    if key in _cache:
        return _cache[key]
    nc = bacc.Bacc(trn_type="TRN2", target_bir_lowering=False, debug=False)

    xt = nc.dram_tensor("xt", [C, T], F16, kind="ExternalInput").ap()
    wq = nc.dram_tensor("wq", [C, HPC], F16, kind="ExternalInput").ap()
    wk = nc.dram_tensor("wk", [C, HPC], F16, kind="ExternalInput").ap()
    wv = nc.dram_tensor("wv", [C, HPC], F16, kind="ExternalInput").ap()
    wo = nc.dram_tensor("wo", [HPC, C], F16, kind="ExternalInput").ap()
    bqk = nc.dram_tensor("bqk", [128, 2 * NHP], F32, kind="ExternalInput").ap()
    bv = nc.dram_tensor("bv", [128, HPC], F16, kind="ExternalInput").ap()
    masks = nc.dram_tensor("masks", [128, 4 * 512], F16, kind="ExternalInput").ap()
    yt = nc.dram_tensor("yt", [C, T], F32, kind="ExternalOutput").ap()

    with tile.TileContext(nc) as tc:
        with (
            tc.tile_pool(name="wp", bufs=1) as wp,          # persistent weights/consts
            tc.tile_pool(name="big", bufs=1) as big,        # QT/KT/V/OT persistent
            tc.tile_pool(name="xs", bufs=2) as xs,          # streamed xT chunks
            tc.tile_pool(name="ev", bufs=3) as ev,          # small sbuf staging
            tc.tile_pool(name="ps", bufs=1, space="PSUM") as ps,
        ):
            # ---- persistent loads (outside the timing loop) ----
            wq_t, wk_t, wv_t = [], [], []
            for c in range(NC_CHUNKS):
                wqc = wp.tile([128, HPC], F16, name=f"wq{c}", tag=f"wq{c}")
                nc.sync.dma_start(wqc[:], wq[c * 128:(c + 1) * 128, :])
                wq_t.append(wqc)
                wkc = wp.tile([128, HPC], F16, name=f"wk{c}", tag=f"wk{c}")
                nc.sync.dma_start(wkc[:], wk[c * 128:(c + 1) * 128, :])
                wk_t.append(wkc)
                wvc = wp.tile([128, HPC], F16, name=f"wv{c}", tag=f"wv{c}")
                nc.sync.dma_start(wvc[:], wv[c * 128:(c + 1) * 128, :])
                wv_t.append(wvc)
            bqk_t = wp.tile([128, 2 * NHP], F32, name="bqk_t", tag="bqk")
            nc.sync.dma_start(bqk_t[:], bqk)
            bv_t = wp.tile([128, HPC], F16, name="bv_t", tag="bv")
            nc.sync.dma_start(bv_t[:], bv)
            mask_t = wp.tile([128, 4 * 512], F16, name="mask_t", tag="mask")
            nc.sync.dma_start(mask_t[:], masks)
            ones64 = wp.tile([1, 64], F32, name="ones64", tag="ones")
            nc.vector.memset(ones64[:], 1.0)
            wo_t = []
            for hp in range(NHP):
                woc = wp.tile([128, C], F16, name=f"wo{hp}", tag=f"wo{hp}")
                nc.sync.dma_start(woc[:], wo[hp * 128:(hp + 1) * 128, :])
                wo_t.append(woc)

            qt_t = [big.tile([128, T], F16, name=f"qt{i}", tag=f"qt{i}") for i in range(NHP)]
            kt_t = [big.tile([128, T], F16, name=f"kt{i}", tag=f"kt{i}") for i in range(NHP)]
            v_t = [big.tile([128, 8, 65], F16, name=f"v{i}", tag=f"v{i}") for i in range(NTB)]
            ot_t = [big.tile([128, T], F16, name=f"ot{i}", tag=f"ot{i}") for i in range(NHP)]

            def body():
                # ---- phase 1: QKV projection ----
                for ts in range(NTS):
                    xc = []
                    for c in range(NC_CHUNKS):
                        x_ts = xs.tile([128, 512], F16, name=f"x_{ts}_{c}", tag=f"x{c}")
                        nc.sync.dma_start(
                            x_ts[:], xt[c * 128:(c + 1) * 128, ts * 512:(ts + 1) * 512])
                        xc.append(x_ts)
                    for hp in range(NHP):
                        pq = ps.tile([128, 512], F32, name=f"pq_{ts}_{hp}", tag="pq", bufs=2)
                        for c in range(NC_CHUNKS):
                            nc.tensor.matmul(
                                pq[:], wq_t[c][:, hp * 128:(hp + 1) * 128], xc[c][:],
                                start=(c == 0), stop=(c == NC_CHUNKS - 1),
                            )
                        nc.scalar.activation(
                            qt_t[hp][:, ts * 512:(ts + 1) * 512], pq[:], AF.Identity,
                            bias=bqk_t[:, hp:hp + 1],
                        )
                        pk = ps.tile([128, 512], F32, name=f"pk_{ts}_{hp}", tag="pq", bufs=2)
                        for c in range(NC_CHUNKS):
                            nc.tensor.matmul(
                                pk[:], wk_t[c][:, hp * 128:(hp + 1) * 128], xc[c][:],
                                start=(c == 0), stop=(c == NC_CHUNKS - 1),
                            )
                        nc.scalar.activation(
                            kt_t[hp][:, ts * 512:(ts + 1) * 512], pk[:], AF.Identity,
                            bias=bqk_t[:, NHP + hp:NHP + hp + 1],
                        )
                    for tb in range(4):
                        pv = ps.tile([128, 512], F32, name=f"pv_{ts}_{tb}", tag="pq", bufs=2)
                        for c in range(NC_CHUNKS):
                            nc.tensor.matmul(
                                pv[:], xc[c][:, tb * 128:(tb + 1) * 128], wv_t[c][:],
                                start=(c == 0), stop=(c == NC_CHUNKS - 1),
                            )
                        vt = v_t[ts * 4 + tb]
                        nc.vector.tensor_add(
                            vt[:, :, 0:64],
                            pv[:].rearrange("p (h d) -> p h d", h=8),
                            bv_t[:].rearrange("p (h d) -> p h d", h=8),
                        )
                        nc.vector.memset(vt[:, :, 64:65], 1.0)

                # ---- phase 2: causal attention ----
                for h in range(8):
                    hp, off = h // 2, 64 * (h % 2)
                    for j in range(NTS):
                        ot = ps.tile([65, 512], F32, name=f"ot_{h}_{j}", tag="ot", bufs=2)
                        nkb = 4 * j + 4
                        for kb in range(nkb):
                            st = ps.tile([128, 512], F32, name=f"st_{h}_{j}_{kb}", tag="st", bufs=2)
                            nc.tensor.matmul(
                                st[:],
                                kt_t[hp][off:off + 64, kb * 128:(kb + 1) * 128],
                                qt_t[hp][off:off + 64, j * 512:(j + 1) * 512],
                                start=True, stop=True,
                            )
                            p16 = ev.tile([128, 512], F16, name=f"p_{h}_{j}_{kb}", tag="p", bufs=3)
                            r = kb - 4 * j
                            if r >= 0:
                                praw = ev.tile(
                                    [128, 512], F16, name=f"pr_{h}_{j}_{kb}", tag="praw", bufs=2)
                                nc.scalar.activation(praw[:], st[:], AF.Exp, scale=SCALE)
                                nc.vector.tensor_mul(
                                    p16[:], praw[:], mask_t[:, r * 512:(r + 1) * 512]
                                )
                            else:
                                nc.scalar.activation(p16[:], st[:], AF.Exp, scale=SCALE)
                            nc.tensor.matmul(
                                ot[:], v_t[kb][:, h, :], p16[:],
                                start=(kb == 0), stop=(kb == nkb - 1),
                            )
                        recip = ev.tile([1, 512], F32, name=f"rc_{h}_{j}", tag="recip", bufs=2)
                        nc.vector.reciprocal(recip[:], ot[64:65, :])
                        bc_ps = ps.tile([64, 512], F32, name=f"bc_{h}_{j}", tag="bc", bufs=1)
                        nc.tensor.matmul(bc_ps[:], ones64[:], recip[:], start=True, stop=True)
                        bc_sb = ev.tile([64, 512], F32, name=f"bs_{h}_{j}", tag="bcs", bufs=2)
                        nc.scalar.activation(bc_sb[:], bc_ps[:], AF.Copy)
                        nc.vector.tensor_mul(
                            ot_t[hp][off:off + 64, j * 512:(j + 1) * 512],
                            ot[0:64, :], bc_sb[:],
                        )

                # ---- phase 3: output projection (partial) ----
                for cc in range(C // 128):
                    for qs in range(NTS):
                        py = ps.tile([128, 512], F32, name=f"py_{cc}_{qs}", tag="pq", bufs=2)
                        for hp in range(NHP):
                            nc.tensor.matmul(
                                py[:],
                                wo_t[hp][:, cc * 128:(cc + 1) * 128],
                                ot_t[hp][:, qs * 512:(qs + 1) * 512],
                                start=(hp == 0), stop=(hp == NHP - 1),
                            )
                        ys = ev.tile([128, 512], F32, name=f"ys_{cc}_{qs}", tag="ys", bufs=3)
                        nc.scalar.activation(ys[:], py[:], AF.Copy)
                        nc.sync.dma_start(
                            yt[cc * 128:(cc + 1) * 128, qs * 512:(qs + 1) * 512], ys[:]
                        )

            if loop_iters > 1:
                with tc.For_i(0, loop_iters, 1):
                    body()
            else:
                body()

    nc.compile()
    _cache[key] = nc
    return nc


def _make_masks():
    kk = np.arange(128)[:, None]
    qq = np.arange(512)[None, :]
    m = np.zeros((128, 4 * 512), dtype=np.float16)
    for r in range(4):
        m[:, r * 512:(r + 1) * 512] = (kk <= qq - 128 * r).astype(np.float16)
    return m


def kernel(x, W_qkv, b_qkv, W_out, b_out):
    x = np.asarray(x, dtype=np.float32)
    W_qkv = np.asarray(W_qkv, dtype=np.float32)
    b_qkv = np.asarray(b_qkv, dtype=np.float32)
    W_out = np.asarray(W_out, dtype=np.float32)
    b_out = np.asarray(b_out, dtype=np.float32)

    nc = _build()
    masks = _make_masks()

    in_maps = []
    for core in range(8):
        b, g = core // 2, core % 2
        sl = slice(g * HPC, (g + 1) * HPC)
        bq_c = b_qkv[0 * C:1 * C][sl]
        bk_c = b_qkv[1 * C:2 * C][sl]
        bv_c = b_qkv[2 * C:3 * C][sl]
        in_maps.append(dict(
            xt=np.ascontiguousarray(x[b].T).astype(np.float16),
            wq=W_qkv[:, 0 * C:1 * C][:, sl].astype(np.float16),
            wk=W_qkv[:, 1 * C:2 * C][:, sl].astype(np.float16),
            wv=W_qkv[:, 2 * C:3 * C][:, sl].astype(np.float16),
            wo=W_out[sl, :].astype(np.float16),
            bqk=np.concatenate(
                [bq_c.reshape(NHP, 128).T, bk_c.reshape(NHP, 128).T], axis=1
            ).astype(np.float32),
            bv=np.tile(bv_c[None, :], (128, 1)).astype(np.float16),
            masks=masks,
        ))

    res = bass_utils.run_bass_kernel_spmd(nc, in_maps, core_ids=list(range(8)))
    out = np.zeros((B, T, C), dtype=np.float32)
    for core in range(8):
        b = core // 2
        out[b] += res.results[core]["yt"].T
    out += b_out[None, None, :]
    return out


# revision 31
# speedup vs baseline: 7.4052x; 7.4052x over previous
"""Causal self-attention TRN2 kernel.

Problem: B=4, T=2048, C=1024, H=16 heads, Dh=64, fp32 I/O.

Sharding: 8 cores = 4 batches x 2 head-groups (8 heads each). Each core
computes QKV projection for its head-group, causal attention, and a partial
output projection; the host sums the two partials per batch and adds b_out.

Per-core layout (all matmul operands fp16; PSUM accumulation fp32):
  - xT [C, T] fp16 (host-transposed), Wq/Wk/Wv [C, 512] fp16, Wo [512, C] fp16
  - QT/KT [512, T] fp16 stored as 4 head-pair tiles [128, T] (partition = 2x64 dh)
  - V [T, 520] fp16 stored as 16 t-block tiles [128, 8, 65]: per head 64 V cols
    + a ones column (the AV matmul's ones column accumulates softmax row-sums)
  - scores computed transposed: ST [keys 128, queries 512] = KTh_blk.T @ QTh_chunk
    so exp(ST) feeds the AV matmul directly (no P transpose needed)
  - AV: OT [65, 512] += Vaug[128, 65].T @ P[128, 512]; row 64 = softmax sums
  - normalize: recip of sum row, partition-broadcast via K=1 fp32 matmul
    (ones[1,64].T @ recip[1,512]), fused into the PSUM->SBUF eviction multiply
  - out proj: yT [C, T] fp32 partial = Wo_hp.T @ OT_hp accumulated over head pairs

loop_iters>1 wraps the whole body in a tc.For_i hardware loop (timing only).
"""

import numpy as np

import concourse.bacc as bacc
import concourse.mybir as mybir
import concourse.tile as tile
from concourse import bass_utils

F32 = mybir.dt.float32
F16 = mybir.dt.float16
AF = mybir.ActivationFunctionType

B, T, C = 4, 2048, 1024
H, DH = 16, 64
HPC = 512          # head dims per core (8 heads x 64)
NHP = 4            # head pairs per core
NC_CHUNKS = C // 128   # 8 contraction chunks
NTS = T // 512     # 4 t-chunks of 512
NTB = T // 128     # 16 t-blocks of 128
SCALE = 1.0 / np.sqrt(DH)

_cache = {}


def _build(loop_iters=1, loop_phases=(1, 2, 3), ablate=None):
    key = ("nc", loop_iters, tuple(loop_phases), ablate)
    if key in _cache:
        return _cache[key]
    nc = bacc.Bacc(trn_type="TRN2", target_bir_lowering=False, debug=False)

    xt = nc.dram_tensor("xt", [C, T], F16, kind="ExternalInput").ap()
    wq = nc.dram_tensor("wq", [C, HPC], F16, kind="ExternalInput").ap()
    wk = nc.dram_tensor("wk", [C, HPC], F16, kind="ExternalInput").ap()
    wv = nc.dram_tensor("wv", [C, HPC], F16, kind="ExternalInput").ap()
    wo = nc.dram_tensor("wo", [HPC, C], F16, kind="ExternalInput").ap()
    bqk = nc.dram_tensor("bqk", [128, 2 * NHP], F32, kind="ExternalInput").ap()
    bv = nc.dram_tensor("bv", [128, HPC], F16, kind="ExternalInput").ap()
    masks = nc.dram_tensor("masks", [128, 4 * 512], F16, kind="ExternalInput").ap()
    yt = nc.dram_tensor("yt", [C, T], F32, kind="ExternalOutput").ap()

    with tile.TileContext(nc) as tc:
        with (
            tc.tile_pool(name="wp", bufs=1) as wp,          # persistent weights/consts
            tc.tile_pool(name="big", bufs=1) as big,        # QT/KT/V/OT persistent
            tc.tile_pool(name="xs", bufs=2) as xs,          # streamed xT chunks
            tc.tile_pool(name="ev", bufs=3) as ev,          # small sbuf staging
            tc.tile_pool(name="ps", bufs=1, space="PSUM") as ps,
        ):
            # ---- persistent loads (outside the timing loop) ----
            wq_t, wk_t, wv_t = [], [], []
            for c in range(NC_CHUNKS):
                wqc = wp.tile([128, HPC], F16, name=f"wq{c}", tag=f"wq{c}")
                nc.sync.dma_start(wqc[:], wq[c * 128:(c + 1) * 128, :])
                wq_t.append(wqc)
                wkc = wp.tile([128, HPC], F16, name=f"wk{c}", tag=f"wk{c}")
                nc.sync.dma_start(wkc[:], wk[c * 128:(c + 1) * 128, :])
                wk_t.append(wkc)
                wvc = wp.tile([128, HPC], F16, name=f"wv{c}", tag=f"wv{c}")
                nc.sync.dma_start(wvc[:], wv[c * 128:(c + 1) * 128, :])
                wv_t.append(wvc)
            bqk_t = wp.tile([128, 2 * NHP], F32, name="bqk_t", tag="bqk")
            nc.sync.dma_start(bqk_t[:], bqk)
            bv_t = wp.tile([128, HPC], F16, name="bv_t", tag="bv")
            nc.sync.dma_start(bv_t[:], bv)
            mask_t = wp.tile([128, 4 * 512], F16, name="mask_t", tag="mask")
            nc.sync.dma_start(mask_t[:], masks)
            ones64 = wp.tile([1, 64], F32, name="ones64", tag="ones")
            nc.vector.memset(ones64[:], 1.0)
            wo_t = []
            for hp in range(NHP):
                woc = wp.tile([128, C], F16, name=f"wo{hp}", tag=f"wo{hp}")
                nc.sync.dma_start(woc[:], wo[hp * 128:(hp + 1) * 128, :])
                wo_t.append(woc)
            if ablate is not None:
                pconst = wp.tile([128, 1024], F16, name="pconst", tag="pconst")
                nc.vector.memset(pconst[:], 2.0 ** -11)
                bcdummy = wp.tile([64, 1024], F32, name="bcdummy", tag="bcdummy")
                nc.vector.memset(bcdummy[:], 1.0)

            qt_t = [big.tile([128, T], F16, name=f"qt{i}", tag=f"qt{i}") for i in range(NHP)]
            kt_t = [big.tile([128, T], F16, name=f"kt{i}", tag=f"kt{i}") for i in range(NHP)]
            v_t = [big.tile([128, 8, 65], F16, name=f"v{i}", tag=f"v{i}") for i in range(NTB)]
            ot_t = [big.tile([128, T], F16, name=f"ot{i}", tag=f"ot{i}") for i in range(NHP)]

            def body(phases=(1, 2, 3), real=False):
                if 1 in phases:
                    phase1()
                if 2 in phases:
                    phase2(real=real)
                if 3 in phases:
                    phase3()

            def phase1():
                # ---- phase 1: QKV projection ----
                for ts in range(NTS):
                    xc = []
                    for c in range(NC_CHUNKS):
                        x_ts = xs.tile([128, 512], F16, name=f"x_{ts}_{c}", tag=f"x{c}")
                        nc.sync.dma_start(
                            x_ts[:], xt[c * 128:(c + 1) * 128, ts * 512:(ts + 1) * 512])
                        xc.append(x_ts)
                    for hp in range(NHP):
                        pq = ps.tile([128, 512], F32, name=f"pq_{ts}_{hp}", tag="st", bufs=2)
                        for c in range(NC_CHUNKS):
                            nc.tensor.matmul(
                                pq[:], wq_t[c][:, hp * 128:(hp + 1) * 128], xc[c][:],
                                start=(c == 0), stop=(c == NC_CHUNKS - 1),
                            )
                        nc.scalar.activation(
                            qt_t[hp][:, ts * 512:(ts + 1) * 512], pq[:], AF.Identity,
                            bias=bqk_t[:, hp:hp + 1],
                        )
                        pk = ps.tile([128, 512], F32, name=f"pk_{ts}_{hp}", tag="st", bufs=2)
                        for c in range(NC_CHUNKS):
                            nc.tensor.matmul(
                                pk[:], wk_t[c][:, hp * 128:(hp + 1) * 128], xc[c][:],
                                start=(c == 0), stop=(c == NC_CHUNKS - 1),
                            )
                        nc.scalar.activation(
                            kt_t[hp][:, ts * 512:(ts + 1) * 512], pk[:], AF.Identity,
                            bias=bqk_t[:, NHP + hp:NHP + hp + 1],
                        )
                    for tb in range(4):
                        pv = ps.tile([128, 512], F32, name=f"pv_{ts}_{tb}", tag="st", bufs=2)
                        for c in range(NC_CHUNKS):
                            nc.tensor.matmul(
                                pv[:], xc[c][:, tb * 128:(tb + 1) * 128], wv_t[c][:],
                                start=(c == 0), stop=(c == NC_CHUNKS - 1),
                            )
                        vt = v_t[ts * 4 + tb]
                        nc.vector.tensor_add(
                            vt[:, :, 0:64],
                            pv[:].rearrange("p (h d) -> p h d", h=8),
                            bv_t[:].rearrange("p (h d) -> p h d", h=8),
                        )
                        nc.vector.memset(vt[:, :, 64:65], 1.0)

            def phase2(real=False):
                if not real and ablate in ("mm512", "mm512acc", "mmst", "mmav"):
                    # PE micro-benchmarks: 640 matmuls in phase-2's slot
                    for i in range(640):
                        st = ps.tile([128, 512], F32, name=f"mb_{i}", tag="st", bufs=2)
                        if ablate == "mm512":
                            nc.tensor.matmul(st[:], wq_t[0][:, 0:128], pconst[:, 0:512],
                                             start=True, stop=True)
                        elif ablate == "mm512acc":
                            nc.tensor.matmul(st[:], wq_t[0][:, 0:128], pconst[:, 0:512],
                                             start=(i % 8 == 0), stop=(i % 8 == 7))
                        elif ablate == "mmst":
                            nc.tensor.matmul(st[:], kt_t[0][0:64, 0:128],
                                             qt_t[0][0:64, 0:512], start=True, stop=True)
                        else:  # mmav
                            nc.tensor.matmul(st[0:65, :], v_t[i % 16][:, 0, :],
                                             pconst[:, 0:512], start=True, stop=True)
                    return
                # ---- phase 2: causal attention (j-major for phase-3 overlap) ----
                use_abl = (not real) and ablate in ("nonorm", "dumbc")

                def do_norm(ot, hp, off, j, h):
                    # single DVE copy frees the ot PSUM slot; the rest of the
                    # chain (recip -> gpsimd bcast -> mul) runs from SBUF and
                    # never gates the PE's AV stream
                    oraw = ev.tile([65, 512], F32, name=f"or_{h}_{j}", tag="oraw", bufs=3)
                    nc.vector.tensor_copy(oraw[:], ot[:])
                    dst = ot_t[hp][off:off + 64, j * 512:(j + 1) * 512]
                    if use_abl and ablate == "nonorm":
                        nc.vector.tensor_copy(dst, oraw[0:64, :])
                        return
                    recip = ev.tile([1, 512], F32, name=f"rc_{h}_{j}", tag="recip", bufs=3)
                    nc.vector.reciprocal(recip[:], oraw[64:65, :])
                    if use_abl and ablate == "dumbc":
                        bc = bcdummy
                    else:
                        bc = ev.tile([64, 512], F32, name=f"bs_{h}_{j}", tag="bcs", bufs=3)
                        nc.gpsimd.partition_broadcast(bc[:], recip[:])
                    nc.vector.tensor_mul(dst, oraw[0:64, :], bc[:, :])

                norm_q = []
                pending = None  # deferred AV pair
                otA = None
                for h in range(8):
                    hp, off = h // 2, 64 * (h % 2)
                    for j in range(NTS):
                        nkb = 4 * j + 4
                        ot = ps.tile([65, 512], F32, name=f"ot_{h}_{j}", tag="ot", bufs=2)
                        for m in range(nkb // 2):   # kb pairs
                            kb0, kb1 = 2 * m, 2 * m + 1
                            st = ps.tile([128, 1024], F32, name=f"st_{h}_{j}_{m}", tag="st", bufs=2)
                            for i, kb in enumerate((kb0, kb1)):
                                nc.tensor.matmul(
                                    st[:, i * 512:(i + 1) * 512],
                                    kt_t[hp][off:off + 64, kb * 128:(kb + 1) * 128],
                                    qt_t[hp][off:off + 64, j * 512:(j + 1) * 512],
                                    start=True, stop=True,
                                )
                            if (not real) and ablate == "noexp":
                                p16 = pconst
                            else:
                                p16 = ev.tile(
                                    [128, 1024], F16, name=f"p_{h}_{j}_{m}", tag="p", bufs=4)
                                if kb1 >= 4 * j:  # pair contains diagonal-masked blocks
                                    r = kb0 - 4 * j  # 0 or 2
                                    praw = ev.tile(
                                        [128, 1024], F16, name=f"pr_{h}_{j}_{m}", tag="praw", bufs=3)
                                    nc.scalar.activation(praw[:], st[:], AF.Exp, scale=SCALE)
                                    nc.vector.tensor_mul(
                                        p16[:], praw[:], mask_t[:, r * 512:(r + 2) * 512]
                                    )
                                else:
                                    nc.scalar.activation(p16[:], st[:], AF.Exp, scale=SCALE)
                            if pending is not None:
                                pot, ph, pkb0, pkb1, pp16, pnkb = pending
                                for i, kb in enumerate((pkb0, pkb1)):
                                    nc.tensor.matmul(
                                        pot[:], v_t[kb][:, ph, :], pp16[:, i * 512:(i + 1) * 512],
                                        start=(kb == 0), stop=(kb == pnkb - 1),
                                    )
                            pending = (ot, h, kb0, kb1, p16, nkb)
                        norm_q.append((ot, hp, off, j, h))
                        if len(norm_q) >= 2:
                            do_norm(*norm_q.pop(0))
                if pending is not None:
                    pot, ph, pkb0, pkb1, pp16, pnkb = pending
                    for i, kb in enumerate((pkb0, pkb1)):
                        nc.tensor.matmul(
                            pot[:], v_t[kb][:, ph, :], pp16[:, i * 512:(i + 1) * 512],
                            start=(kb == 0), stop=(kb == pnkb - 1),
                        )
                    pending = None
                for args in norm_q:
                    do_norm(*args)

            def phase3():
                # ---- phase 3: output projection (partial) ----
                for cc in range(C // 128):
                    for qs in range(NTS):
                        py = ps.tile([128, 512], F32, name=f"py_{cc}_{qs}", tag="st", bufs=2)
                        for hp in range(NHP):
                            nc.tensor.matmul(
                                py[:],
                                wo_t[hp][:, cc * 128:(cc + 1) * 128],
                                ot_t[hp][:, qs * 512:(qs + 1) * 512],
                                start=(hp == 0), stop=(hp == NHP - 1),
                            )
                        ys = ev.tile([128, 512], F32, name=f"ys_{cc}_{qs}", tag="ys", bufs=3)
                        nc.scalar.activation(ys[:], py[:], AF.Copy)
                        nc.sync.dma_start(
                            yt[cc * 128:(cc + 1) * 128, qs * 512:(qs + 1) * 512], ys[:]
                        )

            if loop_iters > 1:
                if tuple(loop_phases) != (1, 2, 3) or ablate is not None:
                    body(real=True)  # populate intermediates once
                with tc.For_i(0, loop_iters, 1):
                    body(tuple(loop_phases))
            else:
                body()

    nc.compile()
    _cache[key] = nc
    return nc


def _make_masks():
    kk = np.arange(128)[:, None]
    qq = np.arange(512)[None, :]
    m = np.zeros((128, 4 * 512), dtype=np.float16)
    for r in range(4):
        m[:, r * 512:(r + 1) * 512] = (kk <= qq - 128 * r).astype(np.float16)
    return m


def kernel(x, W_qkv, b_qkv, W_out, b_out):
    x = np.asarray(x, dtype=np.float32)
    W_qkv = np.asarray(W_qkv, dtype=np.float32)
    b_qkv = np.asarray(b_qkv, dtype=np.float32)
    W_out = np.asarray(W_out, dtype=np.float32)
    b_out = np.asarray(b_out, dtype=np.float32)

    nc = _build()
    masks = _make_masks()

    in_maps = []
    for core in range(8):
        b, g = core // 2, core % 2
        sl = slice(g * HPC, (g + 1) * HPC)
        bq_c = b_qkv[0 * C:1 * C][sl]
        bk_c = b_qkv[1 * C:2 * C][sl]
        bv_c = b_qkv[2 * C:3 * C][sl]
        in_maps.append(dict(
            xt=np.ascontiguousarray(x[b].T).astype(np.float16),
            wq=W_qkv[:, 0 * C:1 * C][:, sl].astype(np.float16),
            wk=W_qkv[:, 1 * C:2 * C][:, sl].astype(np.float16),
            wv=W_qkv[:, 2 * C:3 * C][:, sl].astype(np.float16),
            wo=W_out[sl, :].astype(np.float16),
            bqk=np.concatenate(
                [bq_c.reshape(NHP, 128).T, bk_c.reshape(NHP, 128).T], axis=1
            ).astype(np.float32),
            bv=np.tile(bv_c[None, :], (128, 1)).astype(np.float16),
            masks=masks,
        ))

    res = bass_utils.run_bass_kernel_spmd(nc, in_maps, core_ids=list(range(8)))
    out = np.zeros((B, T, C), dtype=np.float32)
    for core in range(8):
        b = core // 2
        out[b] += res.results[core]["yt"].T
    out += b_out[None, None, :]
    return out
